# revision 1
# baseline (speedup 1.0000x reference)
"""Dynamic structural masking attention on 8 Trainium2 NeuronCores.

Reference computation (per batch b):
    sim  = cos_sim(x, x)                      [S, S]
    mask = sim > 0.7                          (shared across heads)
    q/k/v = x @ W.T + b, per-head split
    out  = softmax(where(mask, q k^T / 8, -inf)) @ v   [H, S, dk]

Sharding over 8 cores: batch (2) x head-group (2) x query-slice (2).
Each core computes, for its batch b, 8 heads and 1024 query rows:
  - Gram matrix G = x^T x rows for its queries (transposed layout), with
    norms folded into the threshold compare -> 0/1 mask tile (bf16).
    The query-block of the mask is symmetric; below-diagonal 512-spans
    are filled by bf16 xbar DMA-transposes instead of matmuls.
  - Projections QT/KT (transposed, bf16) and V (normal layout, fp32r)
    augmented with a ones column so the attention-weight row-sum
    (softmax denominator) falls out of the AV matmul for free.
  - Per head: scores^T = K Q^T per t-chunk, exp on ACT (scale=1/8),
    mask multiply on DVE, AV accumulate on PE (emission software-
    pipelined: AV lags scores; epilogues interleave into the next head;
    the K-projection is braided into this phase to fill PE idle time).
    The epilogue normalizes in transposed layout (reciprocal of the
    PSUM denominator row, GPSIMD partition-broadcast, one DVE multiply)
    and the host de-transposes the [dk, q] output slabs.

Matmuls run in fp32r (full PE rate at N>=256, ~1.5e-4 rel precision);
Q/K and scores use bf16 (their precision only shifts softmax weights
within the masked set). Key order per core is permuted so its query
slice occupies columns 0:SQ (attention is permutation-invariant over
keys) - the same SPMD program serves all cores with no dynamic offsets.
Cost-model timeline: ~311 us/core; verified vs the fp64 reference at
2.1e-4 max relative error on hardware.
"""

import numpy as np

# Problem dims (hardcoded per contract; kernel.py must be self-contained).
B = 2
S = 2048
D = 1024
H_TOT = 16
DK = 64
SIM_THRESH = 0.7
N_CORES = 8

_CACHE = {}


def _build(S_, D_, H_LOC, SQ, thresh, n_cores=N_CORES, debug_mask=False):
    """Build + compile the SPMD single-core program."""
    import concourse.bacc as bacc
    import concourse.mybir as mybir
    import concourse.tile as tile

    f32 = mybir.dt.float32
    f32r = mybir.dt.float32r
    bf16 = mybir.dt.bfloat16
    Alu = mybir.AluOpType
    Act = mybir.ActivationFunctionType

    JH = H_LOC * DK          # projection output cols per core
    ND = D_ // 128           # contraction chunks
    NT = S_ // 128           # key chunks
    NQ = SQ // 128           # query chunks
    NJ = JH // 128           # projection col chunks
    NSP = SQ // 512          # 512-wide spans over queries
    NKS = S_ // 512          # 512-wide spans over keys
    HPJ = 128 // DK          # heads per j-chunk
    assert SQ % 512 == 0 and S_ % 1024 == 0 and JH % 128 == 0

    nc = bacc.Bacc("TRN2", target_bir_lowering=False, debug=False,
                   num_devices=n_cores)

    xt_d = nc.dram_tensor("xt", [D_, S_], f32, kind="ExternalInput")
    wqt_d = nc.dram_tensor("wqt", [D_, JH], f32, kind="ExternalInput")
    wkt_d = nc.dram_tensor("wkt", [D_, JH], f32, kind="ExternalInput")
    wvt_d = nc.dram_tensor("wvt", [D_, JH], f32, kind="ExternalInput")
    bq_d = nc.dram_tensor("bq", [JH], f32, kind="ExternalInput")
    bk_d = nc.dram_tensor("bk", [JH], f32, kind="ExternalInput")
    bvb_d = nc.dram_tensor("bvb", [128, JH], f32, kind="ExternalInput")
    ones_d = nc.dram_tensor("ones1", [128, 1], f32, kind="ExternalInput")
    out_d = nc.dram_tensor("out", [H_LOC, DK, SQ], f32, kind="ExternalOutput")
    maskout_d = None
    if debug_mask:
        maskout_d = nc.dram_tensor("maskout", [S_, SQ], mybir.dt.bfloat16,
                                   kind="ExternalOutput")

    with tile.TileContext(nc) as tc:
        with (
            tc.tile_pool(name="small", bufs=1) as small,
            tc.tile_pool(name="mask", bufs=NT) as mask_pool,
            tc.tile_pool(name="qt", bufs=NJ) as qt_pool,
            tc.tile_pool(name="kt", bufs=NJ) as kt_pool,
            tc.tile_pool(name="vp", bufs=NT) as v_pool,
            tc.tile_pool(name="dram", bufs=1, space="DRAM") as dram,
        ):
            # --- persistent small tiles ---
            ones_t = small.tile([128, 1], f32r, tag="ones")
            nc.gpsimd.dma_start(ones_t[:], ones_d.ap())
            bq_t = small.tile([128, NJ], f32, tag="bq")
            nc.sync.dma_start(bq_t[:], bq_d.ap().rearrange("(c p) -> p c", p=128))
            bk_t = small.tile([128, NJ], f32, tag="bk")
            nc.sync.dma_start(bk_t[:], bk_d.ap().rearrange("(c p) -> p c", p=128))
            bvb_t = small.tile([128, JH], f32, tag="bvb")
            nc.sync.dma_start(bvb_t[:], bvb_d.ap())
            ones8_t = small.tile([128, H_LOC], f32, tag="ones8")
            nc.vector.memset(ones8_t[:], 1.0)
            dscr = dram.tile([1, S_], f32, tag="dscr")

            mask_t = [mask_pool.tile([128, SQ], bf16, tag="mask", name=f"mask{i}") for i in range(NT)]
            qt_t = [qt_pool.tile([128, SQ], bf16, tag="qt", name=f"qt{i}") for i in range(NJ)]
            kt_t = [kt_pool.tile([128, S_], bf16, tag="kt", name=f"kt{i}") for i in range(NJ)]
            v_t = [v_pool.tile([128, H_LOC, 65], f32r, tag="v", name=f"v{i}") for i in range(NT)]

            with tc.tile_pool(name="xt", bufs=ND) as xt_pool:
              with (
                tc.tile_pool(name="thr", bufs=1) as thr_pool,
                tc.tile_pool(name="ps", bufs=3, space="PSUM") as ps,
              ):
                xt_t = [xt_pool.tile([128, S_], f32r, tag="xt", name=f"xtt{i}") for i in range(ND)]
                # query-slice columns first: G/QT/norm matmuls depend only on
                # cols 0:SQ plus each t-chunk's own columns, so PE starts as
                # soon as the first-half DMAs land
                for dc in range(ND):
                    nc.gpsimd.dma_start(xt_t[dc][:, 0:SQ],
                                        xt_d.ap()[dc * 128:(dc + 1) * 128, 0:SQ])
                if SQ < S_:
                    for dc in range(ND):
                        nc.gpsimd.dma_start(xt_t[dc][:, SQ:S_],
                                            xt_d.ap()[dc * 128:(dc + 1) * 128, SQ:S_])

                thrq_bc = thr_pool.tile([128, SQ], f32, tag="thrqbc")
                invnk_cols = thr_pool.tile([128, NT], f32, tag="invnkcols")

                # --- stage A: key norms via squares + ones-matmul reduce ---
                # processed in 1024-key groups so the first mask compares only
                # wait on first-half norms (second-half xt arrives later)
                with tc.tile_pool(name="sta", bufs=1) as sta:
                    nk_row = sta.tile([1, S_], f32, tag="nkrow")
                    thrq_row = sta.tile([1, SQ], f32, tag="thrqrow")
                    with tc.tile_pool(name="sqtmp", bufs=3) as sqp:
                        for grp in range(S_ // 1024):
                            for sp in (2 * grp, 2 * grp + 1):
                                n2_ps = ps.tile([128, 1024], f32, tag="ps")
                                for dc in range(ND):
                                    sq_t = sqp.tile([128, 512], f32r, tag="sq")
                                    nc.scalar.activation(
                                        sq_t[:],
                                        xt_t[dc][:, sp * 512:(sp + 1) * 512].bitcast(f32),
                                        Act.Square)
                                    nc.tensor.matmul(n2_ps[0:1, 0:512], ones_t[:],
                                                     sq_t[:], start=(dc == 0),
                                                     stop=(dc == ND - 1))
                                nc.scalar.activation(
                                    nk_row[0:1, sp * 512:(sp + 1) * 512],
                                    n2_ps[0:1, 0:512], Act.Sqrt)
                                if sp < NSP:
                                    nc.scalar.activation(
                                        thrq_row[0:1, sp * 512:(sp + 1) * 512],
                                        n2_ps[0:1, 0:512], Act.Sqrt,
                                        scale=thresh * thresh)
                            if grp == 0:
                                nc.gpsimd.partition_broadcast(thrq_bc[:], thrq_row[:])
                            a, b = grp * 1024, (grp + 1) * 1024
                            nc.vector.reciprocal(nk_row[0:1, a:b], nk_row[0:1, a:b])
                            nc.sync.dma_start(dscr[0:1, a:b], nk_row[0:1, a:b])
                            nc.sync.dma_start(
                                invnk_cols[:, grp * 8:(grp + 1) * 8],
                                dscr[0:1, a:b].rearrange("o (c p) -> (o p) c", p=128))

                # --- stage B: Gram rows -> mask; Q projection ---
                # The [keys 0:SQ, queries 0:SQ] block of the mask is
                # symmetric (queries are keys 0:SQ in core-local order), so
                # below-diagonal 256-spans are filled by bf16 xbar
                # DMA-transposes of already-computed tiles instead of
                # Gram matmuls.
                NQT = SQ // 128  # tiles whose keys lie in the query slice
                for tcn in range(NT):
                    sav = tcn // 4 if tcn < NQT else 0  # saved 512-spans
                    col0 = sav * 512
                    g_ps = ps.tile([128, 1024], f32, tag="ps")
                    for dc in range(ND):
                        for sp in range((SQ - col0) // 512):
                            a = col0 + sp * 512
                            nc.tensor.matmul(
                                g_ps[:, a:a + 512],
                                xt_t[dc][:, tcn * 128:(tcn + 1) * 128],
                                xt_t[dc][:, a:a + 512],
                                start=(dc == 0), stop=(dc == ND - 1))
                    # mask[k, q] = (G * (1/|x_k|)) > 0.7*|x_q|
                    nc.vector.scalar_tensor_tensor(
                        mask_t[tcn][:, col0:SQ], g_ps[:, col0:SQ],
                        invnk_cols[:, tcn:tcn + 1],
                        thrq_bc[:, col0:SQ], op0=Alu.mult, op1=Alu.is_gt)
                    for m in range(4 * sav):
                        nc.sync.dma_start(
                            mask_t[tcn][:, m * 128:(m + 1) * 128],
                            mask_t[m][:, tcn * 128:(tcn + 1) * 128],
                            transpose=True)
                    if maskout_d is not None:
                        nc.sync.dma_start(
                            maskout_d.ap()[tcn * 128:(tcn + 1) * 128, :],
                            mask_t[tcn][:])

                with tc.tile_pool(name="wq", bufs=ND) as wqp:
                    wq_c = []
                    for dc in range(ND):
                        wt = wqp.tile([128, JH], f32r, tag="w", name=f"wq{dc}")
                        nc.gpsimd.dma_start(wt[:],
                                            wqt_d.ap()[dc * 128:(dc + 1) * 128, :])
                        wq_c.append(wt)
                    for jc in range(NJ):
                        q_ps = ps.tile([128, 1024], f32, tag="ps")
                        for dc in range(ND):
                            for sp in range(NSP):
                                nc.tensor.matmul(
                                    q_ps[:, sp * 512:(sp + 1) * 512],
                                    wq_c[dc][:, jc * 128:(jc + 1) * 128],
                                    xt_t[dc][:, sp * 512:(sp + 1) * 512],
                                    start=(dc == 0), stop=(dc == ND - 1))
                        nc.scalar.activation(qt_t[jc][:], q_ps[:, 0:SQ], Act.Identity,
                                             bias=bq_t[:, jc:jc + 1])

                # --- stage C: K^T and V projections ---
                with tc.tile_pool(name="wv", bufs=ND) as wvp:
                    wv_c = []
                    for dc in range(ND):
                        wt = wvp.tile([128, JH], f32r, tag="w", name=f"wv{dc}")
                        nc.gpsimd.dma_start(wt[:],
                                            wvt_d.ap()[dc * 128:(dc + 1) * 128, :])
                        wv_c.append(wt)
                    for sc in range(NT):
                        v_ps = ps.tile([128, 1024], f32, tag="ps")
                        for dc in range(ND):
                            nc.tensor.matmul(
                                v_ps[:, 0:JH],
                                xt_t[dc][:, sc * 128:(sc + 1) * 128],
                                wv_c[dc][:],
                                start=(dc == 0), stop=(dc == ND - 1))
                        nc.vector.tensor_tensor(
                            v_t[sc][:, :, 0:64],
                            v_ps[:, 0:JH].rearrange("p (h e) -> p h e", h=H_LOC),
                            bvb_t[:].rearrange("p (h e) -> p h e", h=H_LOC),
                            op=Alu.add)
                        nc.vector.tensor_copy(v_t[sc][:, :, 64], ones8_t[:])

              # --- stage D: per-head masked attention ---
              with (
                  tc.tile_pool(name="p", bufs=9) as p_pool,
                  tc.tile_pool(name="osb", bufs=1) as out_pool,
                  tc.tile_pool(name="rec", bufs=1) as rec_pool,
                  tc.tile_pool(name="bc", bufs=1) as bc_pool,
                  tc.tile_pool(name="wk", bufs=ND + 2) as wkp,
                  tc.tile_pool(name="scps", bufs=2, space="PSUM") as scps,
                  tc.tile_pool(name="avps", bufs=2, space="PSUM") as avps,
              ):
                  def emit_kt(jc):
                      wk_c = []
                      for dc in range(ND):
                          wt = wkp.tile([128, 128], f32r, tag="w",
                                        name=f"wkc{jc}_{dc}")
                          nc.gpsimd.dma_start(
                              wt[:], wkt_d.ap()[dc * 128:(dc + 1) * 128,
                                                jc * 128:(jc + 1) * 128])
                          wk_c.append(wt)
                      for half in range(S_ // 1024):
                          k_ps = scps.tile([128, 1024], f32, tag="sc",
                                           name=f"kps{jc}_{half}")
                          for dc in range(ND):
                              for sp in range(2):
                                  o = half * 1024 + sp * 512
                                  nc.tensor.matmul(
                                      k_ps[:, sp * 512:(sp + 1) * 512],
                                      wk_c[dc][:],
                                      xt_t[dc][:, o:o + 512],
                                      start=(dc == 0), stop=(dc == ND - 1))
                          nc.scalar.activation(
                              kt_t[jc][:, half * 1024:(half + 1) * 1024],
                              k_ps[:], Act.Identity, bias=bk_t[:, jc:jc + 1])
                  av_ps_of = {}

                  LAG = min(8, NT - 1)  # av emission lag (PE never head-blocks)

                  def emit_scores(h, tcn):
                      jc = h // HPJ
                      ho = (h % HPJ) * DK
                      s_ps = scps.tile([128, 1024], f32, tag="sc",
                                       name=f"sps{h}_{tcn}")
                      for sp in range(NSP):
                          nc.tensor.matmul(
                              s_ps[:, sp * 512:(sp + 1) * 512],
                              kt_t[jc][ho:ho + DK, tcn * 128:(tcn + 1) * 128],
                              qt_t[jc][ho:ho + DK, sp * 512:(sp + 1) * 512],
                              start=True, stop=True)
                      p_t = p_pool.tile([128, SQ], f32r, tag="p",
                                        name=f"p{h}_{tcn}")
                      nc.scalar.activation(p_t[:], s_ps[:, 0:SQ], Act.Exp,
                                           scale=0.125)
                      meng = (nc.gpsimd if h == H_LOC - 1 and tcn % 2 == 1
                              else nc.vector)
                      meng.tensor_tensor(p_t[:], p_t[:].bitcast(f32),
                                         mask_t[tcn][:], op=Alu.mult)
                      return p_t

                  def emit_av(h, tcn, p_t):
                      av_ps = av_ps_of[h]
                      for sp in range(NSP):
                          nc.tensor.matmul(
                              av_ps[:, sp * 512:(sp + 1) * 512],
                              v_t[tcn][:, h, :],
                              p_t[:, sp * 512:(sp + 1) * 512],
                              start=(tcn == 0), stop=(tcn == NT - 1))

                  def head_chunks(h, tcns):
                      for tcn in tcns:
                          p_t = emit_scores(h, tcn)
                          pending.append((h, tcn, p_t))
                          while len(pending) > LAG:
                              nc_h, nc_t, nc_p = pending.pop(0)
                              emit_av(nc_h, nc_t, nc_p)

                  def head_epilogue(h):
                      av_ps = av_ps_of.pop(h)
                      rec_row = rec_pool.tile([1, SQ], f32, tag="rec",
                                              name=f"recrow{h}")
                      nc.vector.reciprocal(rec_row[:], av_ps[64:65, :])
                      rec_bc = bc_pool.tile([DK, SQ], f32, tag="bc",
                                            name=f"recbc{h}")
                      nc.gpsimd.partition_broadcast(rec_bc[:], rec_row[:])
                      o_t = out_pool.tile([DK, SQ], f32, tag="o", name=f"o{h}")
                      nc.vector.tensor_tensor(o_t[:], av_ps[0:DK, :], rec_bc[:],
                                              op=Alu.mult)
                      nc.sync.dma_start(out_d.ap()[h], o_t[:])

                  # software-pipelined: head h-1's epilogue lands after head h's
                  # first chunks so the PSUM->SBUF copy never stalls ACT
                  pending = []
                  PRO = min(NT, max(LAG + 1, (3 * NT) // 4))
                  for h in range(H_LOC):
                      if h % HPJ == 0:
                          emit_kt(h // HPJ)
                      av_ps_of[h] = avps.tile([65, SQ], f32, tag="av",
                                              name=f"avps{h}")
                      head_chunks(h, range(0, PRO))
                      if h > 0:
                          head_epilogue(h - 1)
                      head_chunks(h, range(PRO, NT))
                  while pending:
                      nc_h, nc_t, nc_p = pending.pop(0)
                      emit_av(nc_h, nc_t, nc_p)
                  head_epilogue(H_LOC - 1)

    nc.compile()
    return nc


def _get_nc():
    key = (S, D, H_TOT, SIM_THRESH)
    if key not in _CACHE:
        _CACHE[key] = _build(S, D, 8, 1024, SIM_THRESH)
    return _CACHE[key]


def make_in_maps(x, Wq, bq, Wk, bk, Wv, bv, h_loc=8, sq=1024, n_cores=N_CORES):
    """Per-core input dicts. Core c: batch, head-group, query-slice; its
    keys are rolled so the query slice comes first."""
    x = np.asarray(x, dtype=np.float32)
    Wq, Wk, Wv = (np.asarray(w, dtype=np.float32) for w in (Wq, Wk, Wv))
    bq, bk, bv = (np.asarray(v_, dtype=np.float32) for v_ in (bq, bk, bv))
    jh = h_loc * DK
    seq = x.shape[1]
    d_model = x.shape[2]
    ones1 = np.ones((128, 1), np.float32)
    n_hg = d_model // jh
    n_qs = seq // sq
    in_maps = []
    for c in range(n_cores):
        b = c // (n_hg * n_qs)
        hg = (c % (n_hg * n_qs)) // n_qs
        qs = c % n_qs
        xb = x[b]
        order = np.concatenate([
            np.arange(qs * sq, (qs + 1) * sq),
            np.delete(np.arange(seq), np.s_[qs * sq:(qs + 1) * sq])])
        in_maps.append({
            "xt": np.ascontiguousarray(xb[order].T),
            "wqt": np.ascontiguousarray(Wq[hg * jh:(hg + 1) * jh].T),
            "wkt": np.ascontiguousarray(Wk[hg * jh:(hg + 1) * jh].T),
            "wvt": np.ascontiguousarray(Wv[hg * jh:(hg + 1) * jh].T),
            "bq": np.ascontiguousarray(bq[hg * jh:(hg + 1) * jh]),
            "bk": np.ascontiguousarray(bk[hg * jh:(hg + 1) * jh]),
            "bvb": np.ascontiguousarray(
                np.broadcast_to(bv[hg * jh:(hg + 1) * jh], (128, jh))),
            "ones1": ones1,
        })
    return in_maps


def assemble(results, h_tot=H_TOT, seq=S, h_loc=8, sq=1024, n_cores=N_CORES):
    n_hg = h_tot // h_loc
    n_qs = seq // sq
    n_b = n_cores // (n_hg * n_qs)
    out = np.empty((n_b, h_tot, seq, DK), np.float32)
    for c in range(n_cores):
        b = c // (n_hg * n_qs)
        hg = (c % (n_hg * n_qs)) // n_qs
        qs = c % n_qs
        out[b, hg * h_loc:(hg + 1) * h_loc, qs * sq:(qs + 1) * sq, :] = \
            results[c]["out"].transpose(0, 2, 1)
    return out


def kernel(x, Wq, bq, Wk, bk, Wv, bv, _trace=False):
    from concourse.bass_utils import run_bass_kernel_spmd
    nc = _get_nc()
    in_maps = make_in_maps(x, Wq, bq, Wk, bk, Wv, bv)
    res = run_bass_kernel_spmd(nc, in_maps, core_ids=list(range(N_CORES)),
                               trace=_trace)
    out = assemble(res.results)
    if _trace:
        return out, res
    return out



# revision 6
# speedup vs baseline: 1.1184x; 1.1184x over previous
"""Dynamic structural masking attention on 8 Trainium2 NeuronCores.

Reference computation (per batch b):
    sim  = cos_sim(x, x)                      [S, S]
    mask = sim > 0.7                          (shared across heads)
    q/k/v = x @ W.T + b, per-head split
    out  = softmax(where(mask, q k^T / 8, -inf)) @ v   [H, S, dk]

Sharding over 8 cores: batch (2) x head-group (2) x query-slice (2); each
core computes 8 heads x 1024 queries over all 2048 keys.

All matmuls run in fp8-e4m3 DoubleRow perf mode (2x128-deep contraction at
0.5 cycles/row = 4x the fp32r MAC rate):
  - Gram/mask: x in fp8; norms from fp8 squares (ones-matmul reduce); the
    [q,q] block's below-diagonal spans filled by bf16 xbar DMA transposes.
  - Q/K projections in single fp8 (score errors cancel row-globally in
    softmax); V in split precision (x_hi*w_hi + x_hi*w_lo + x_lo*w_hi,
    ~bf16 accuracy), + bias via a rank-1 [1,2,...] DoubleRow matmul.
  - Scores K^T Q with dk=64 packed as [32,2] (DMA partition-remap of the
    projection outputs); attention weights p = e^{s/8 - C} (C=5 cancels in
    the softmax normalization) stored fp8.
  - AV pairs (v_hi, v_lo) as the DoubleRow halves with p broadcast across
    halves by a 0-stride AP; the softmax denominator rides along as a ones
    column; a delta*I pass adds delta*(v_hi+v_lo) / delta to num/den so
    singleton-mask rows are exact even when p underflows fp8.
The exp+mask is balanced across three engines: most chunks are a single
DVE scalar_tensor_tensor computing round(1.4427*s + M) saturated to uint8
and bitcast as fp8 (the exp2 exponent-field trick; M = -1.7078 unmasked /
-448 masked built into the mask tiles), the rest get ACT Exp->fp8 plus a
mask multiply on Pool (gpsimd) or DVE. Final normalization (num/den) is
done on the host. Verified on hardware vs the fp32 reference.
"""

import numpy as np

# Problem dims (hardcoded per contract; kernel.py must be self-contained).
B = 2
S = 2048
D = 1024
H_TOT = 16
DK = 64
SIM_THRESH = 0.7
N_CORES = 8

_CACHE = {}

# fp8 exp encoding constants (C = 5.0 global downshift, cancels per-row)
C_SHIFT = 5.0
LOG2E = 1.4426950408889634
B_SOLO = 56.0 - 8.0 * C_SHIFT * LOG2E      # -1.70780 (unmasked add-form)
M_MASKED = -448.0
AFF_SCALE = B_SOLO - M_MASKED              # 446.2922
EXP_BIAS_PSUM = -C_SHIFT                   # ACT path from raw scores
DELTA = 0.015625                           # 2^-6: smallest NORMAL fp8 (PE flushes subnormals)

# stage-D flavor tables (tunable): tcn in ADD_TCNS -> add-form mask,
# chunks there use the single-op DVE stt->uint8 path; other tcns keep the
# 0/1 mask and use ACT exp + multiply on Pool (or DVE for DVEMULT pairs).
ADD_TCNS = frozenset(range(0, 9))
DVEMULT = frozenset((h, 15) for h in (1, 3, 5, 7))


def _build(n_cores=N_CORES):
    import concourse.bacc as bacc
    import concourse.mybir as mybir
    import concourse.tile as tile

    f32 = mybir.dt.float32
    bf16 = mybir.dt.bfloat16
    fp8 = mybir.dt.float8e4
    u8 = mybir.dt.uint8
    Alu = mybir.AluOpType
    Act = mybir.ActivationFunctionType
    DR = mybir.MatmulPerfMode.DoubleRow

    H_LOC = 8
    SQ = 1024
    JH = H_LOC * DK          # 512
    NT = S // 128            # 16 key chunks
    NP = D // 256            # 4 d-pairs
    NJ = JH // 128           # 4 projection col chunks
    NSP = SQ // 512          # 2 query spans
    NKS = S // 512           # 4 key spans

    nc = bacc.Bacc("TRN2", target_bir_lowering=False, debug=False,
                   num_devices=n_cores)

    x8h_d = nc.dram_tensor("x8h", [NP, 128, 2, S], fp8, kind="ExternalInput")
    x8l_d = nc.dram_tensor("x8l", [NP, 128, 2, S], fp8, kind="ExternalInput")
    w8q_d = nc.dram_tensor("w8q", [NP, 128, 2, JH], fp8, kind="ExternalInput")
    w8k_d = nc.dram_tensor("w8k", [NP, 128, 2, JH], fp8, kind="ExternalInput")
    w8vh_d = nc.dram_tensor("w8vh", [NP, 128, 2, JH], fp8, kind="ExternalInput")
    w8vl_d = nc.dram_tensor("w8vl", [NP, 128, 2, JH], fp8, kind="ExternalInput")
    bq_d = nc.dram_tensor("bq128", [128, NJ], f32, kind="ExternalInput")
    bk_d = nc.dram_tensor("bk128", [128, NJ], f32, kind="ExternalInput")
    bv8_d = nc.dram_tensor("bv8", [1, 2, JH], fp8, kind="ExternalInput")
    dd_d = nc.dram_tensor("dd", [128, 2, 128], fp8, kind="ExternalInput")
    out_d = nc.dram_tensor("out", [H_LOC, 65, SQ], f32, kind="ExternalOutput")

    with tile.TileContext(nc) as tc:
        with (
            tc.tile_pool(name="xin", bufs=4) as xin,
            tc.tile_pool(name="wts", bufs=4) as wts,
            tc.tile_pool(name="mask", bufs=16) as mpool,
            tc.tile_pool(name="qk8", bufs=4) as qk8,
            tc.tile_pool(name="vhl", bufs=16) as vpool,
            tc.tile_pool(name="lin", bufs=2) as linp,
            tc.tile_pool(name="p8", bufs=8) as p8pool,
            tc.tile_pool(name="e8", bufs=3) as e8pool,
            tc.tile_pool(name="ot", bufs=2) as otpool,
            tc.tile_pool(name="small", bufs=1) as small,
            tc.tile_pool(name="sqp", bufs=4) as sqp,
            tc.tile_pool(name="dram", bufs=1, space="DRAM") as dram,
            tc.tile_pool(name="scps", bufs=2, space="PSUM") as scps,
            tc.tile_pool(name="avps", bufs=1, space="PSUM") as avps,
            tc.tile_pool(name="aux", bufs=2, space="PSUM") as aux,
        ):
            # ---- persistent tiles ----
            x8h = [xin.tile([128, 2, S], fp8, tag="x8h", name=f"x8h{i}")
                   for i in range(NP)]
            x8l = [xin.tile([128, 2, S], fp8, tag="x8l", name=f"x8l{i}")
                   for i in range(NP)]
            w8q = [wts.tile([128, 2, JH], fp8, tag="w8q", name=f"w8q{i}")
                   for i in range(NP)]
            w8k = [wts.tile([128, 2, JH], fp8, tag="w8k", name=f"w8k{i}")
                   for i in range(NP)]
            w8vh = [wts.tile([128, 2, JH], fp8, tag="w8vh", name=f"w8vh{i}")
                    for i in range(NP)]
            w8vl = [wts.tile([128, 2, JH], fp8, tag="w8vl", name=f"w8vl{i}")
                    for i in range(NP)]
            mask_t = [mpool.tile([128, SQ], bf16, tag="mask", name=f"mask{t}")
                      for t in range(NT)]
            qt8 = [qk8.tile([64, 2, SQ], fp8, tag="qt8", name=f"qt8_{t}")
                   for t in range(4)]
            kt8 = [qk8.tile([64, 2, S], fp8, tag="kt8", name=f"kt8_{t}")
                   for t in range(4)]
            vhl = [vpool.tile([128, 2, H_LOC, 66], fp8, tag="vhl",
                              name=f"vhl{t}") for t in range(NT)]
            bq_t = small.tile([128, NJ], f32, tag="bq")
            bk_t = small.tile([128, NJ], f32, tag="bk")
            bv8_t = small.tile([1, 2, JH], fp8, tag="bv8")
            dd_t = small.tile([128, 2, 128], fp8, tag="dd")
            ones16 = small.tile([128, 2, 16], fp8, tag="ones16")
            onesbv = small.tile([1, 2, 128], fp8, tag="onesbv")
            nk_row = small.tile([1, S], f32, tag="nkrow")
            thrq_row = small.tile([1, SQ], f32, tag="thrqrow")
            invnk_cols = small.tile([128, NT], f32, tag="invnk")
            thrq_bc = small.tile([128, SQ], f32, tag="thrqbc")
            neg448_t = small.tile([128, 1], f32, tag="neg448")
            bias5_t = small.tile([128, 1], f32, tag="bias5")
            sq8 = [sqp.tile([128, 2, S], fp8, tag="sq8", name=f"sq8_{i}")
                   for i in range(NP)]
            dscr = dram.tile([1, S], f32, tag="dscr")

            # ---- input DMAs (x8h first: everything needs it) ----
            for i in range(NP):
                nc.sync.dma_start(x8h[i][:], x8h_d.ap()[i])
            for i in range(NP):
                nc.gpsimd.dma_start(w8q[i][:], w8q_d.ap()[i])
                nc.gpsimd.dma_start(w8k[i][:], w8k_d.ap()[i])
            nc.gpsimd.dma_start(bq_t[:], bq_d.ap())
            nc.gpsimd.dma_start(bk_t[:], bk_d.ap())
            nc.gpsimd.dma_start(bv8_t[:], bv8_d.ap())
            nc.gpsimd.dma_start(dd_t[:], dd_d.ap())
            for i in range(NP):
                nc.gpsimd.dma_start(w8vh[i][:], w8vh_d.ap()[i])
                nc.sync.dma_start(x8l[i][:], x8l_d.ap()[i])
                nc.gpsimd.dma_start(w8vl[i][:], w8vl_d.ap()[i])

            nc.vector.memset(ones16[:], 1.0)
            nc.vector.memset(onesbv[:], 1.0)
            nc.vector.memset(neg448_t[:], M_MASKED)
            nc.vector.memset(bias5_t[:], EXP_BIAS_PSUM)
            for t in range(NT):
                nc.gpsimd.memset(vhl[t][:, 0, :, 64:65], 1.0)
                nc.gpsimd.memset(vhl[t][:, 0, :, 65:66], 0.0)
                nc.gpsimd.memset(vhl[t][:, 1, :, 64:66], 0.0)

            # ---- norms: |x|, 0.7|x|, 1/|x| from fp8 squares ----
            def emit_norms():
                for i in range(NP):
                    nc.scalar.activation(sq8[i][:], x8h[i][:], Act.Square)
                for ks in range(NKS):
                    n_ps = aux.tile([128, 512], f32, tag="aux",
                                    name=f"nps{ks}")
                    for i in range(NP):
                        nc.tensor.matmul(
                            n_ps[0:16, :], ones16[:],
                            x8l[0][:, :, 0:512] if False else
                            sq8[i][:, :, ks * 512:(ks + 1) * 512],
                            start=(i == 0), stop=(i == NP - 1), perf_mode=DR)
                    nc.scalar.activation(
                        nk_row[0:1, ks * 512:(ks + 1) * 512],
                        n_ps[0:1, :], Act.Sqrt)
                    if ks < NSP:
                        nc.scalar.activation(
                            thrq_row[0:1, ks * 512:(ks + 1) * 512],
                            n_ps[0:1, :], Act.Sqrt,
                            scale=SIM_THRESH * SIM_THRESH)
                nc.vector.reciprocal(nk_row[0:1, :], nk_row[0:1, :])
                nc.sync.dma_start(dscr[0:1, :], nk_row[0:1, :])
                nc.sync.dma_start(
                    invnk_cols[:],
                    dscr[0:1, :].rearrange("o (c p) -> (o p) c", p=128))
                nc.gpsimd.partition_broadcast(thrq_bc[:], thrq_row[:])

            # ---- Gram chunk -> mask tile (0/1 or add-form) ----
            def emit_gram(t):
                sav = t // 4 if t < 8 else 0
                col0 = sav * 512
                for a in range(col0, SQ, 512):
                    g_ps = aux.tile([128, 512], f32, tag="aux",
                                    name=f"gps{t}_{a}")
                    for i in range(NP):
                        nc.tensor.matmul(
                            g_ps[:], x8h[i][:, :, t * 128:(t + 1) * 128],
                            x8h[i][:, :, a:a + 512],
                            start=(i == 0), stop=(i == NP - 1), perf_mode=DR)
                    nc.vector.scalar_tensor_tensor(
                        mask_t[t][:, a:a + 512], g_ps[:],
                        invnk_cols[:, t:t + 1], thrq_bc[:, a:a + 512],
                        op0=Alu.mult, op1=Alu.is_gt)
                    if t in ADD_TCNS:
                        nc.scalar.activation(
                            mask_t[t][:, a:a + 512], mask_t[t][:, a:a + 512],
                            Act.Identity, scale=AFF_SCALE, bias=neg448_t[:])
                for m in range(4 * sav):
                    nc.sync.dma_start(
                        mask_t[t][:, m * 128:(m + 1) * 128],
                        mask_t[m][:, t * 128:(t + 1) * 128],
                        transpose=True)

            # ---- Q projection (+ remap to [32,2] layout) ----
            def emit_q(jc):
                qlin = linp.tile([128, SQ], fp8, tag="qlin",
                                 name=f"qlin{jc}")
                for sp in range(NSP):
                    q_ps = aux.tile([128, 512], f32, tag="aux",
                                    name=f"qps{jc}_{sp}")
                    for i in range(NP):
                        nc.tensor.matmul(
                            q_ps[:], w8q[i][:, :, jc * 128:(jc + 1) * 128],
                            x8h[i][:, :, sp * 512:(sp + 1) * 512],
                            start=(i == 0), stop=(i == NP - 1), perf_mode=DR)
                    nc.scalar.activation(
                        qlin[:, sp * 512:(sp + 1) * 512], q_ps[:],
                        Act.Identity, bias=bq_t[:, jc:jc + 1])
                for e in range(2):
                    h = 2 * jc + e
                    for half in range(2):
                        nc.gpsimd.dma_start(
                            qt8[h // 2][32 * (h % 2):32 * (h % 2) + 32,
                                        half, :],
                            qlin[e * 64 + half * 32:e * 64 + half * 32 + 32,
                                 :])

            # ---- K projection (+ remap) ----
            def emit_k(jc):
                klin = linp.tile([128, S], fp8, tag="klin", name=f"klin{jc}")
                for ks in range(NKS):
                    k_ps = aux.tile([128, 512], f32, tag="aux",
                                    name=f"kps{jc}_{ks}")
                    for i in range(NP):
                        nc.tensor.matmul(
                            k_ps[:], w8k[i][:, :, jc * 128:(jc + 1) * 128],
                            x8h[i][:, :, ks * 512:(ks + 1) * 512],
                            start=(i == 0), stop=(i == NP - 1), perf_mode=DR)
                    nc.scalar.activation(
                        klin[:, ks * 512:(ks + 1) * 512], k_ps[:],
                        Act.Identity, bias=bk_t[:, jc:jc + 1])
                for e in range(2):
                    h = 2 * jc + e
                    for half in range(2):
                        nc.gpsimd.dma_start(
                            kt8[h // 2][32 * (h % 2):32 * (h % 2) + 32,
                                        half, :],
                            klin[e * 64 + half * 32:e * 64 + half * 32 + 32,
                                 :])

            # ---- V chunk: hi/lo split with ones column ----
            def emit_v(sc):
                v_ps = aux.tile([128, 512], f32, tag="aux", name=f"vps{sc}")
                first = True
                for xa, wb in ((x8h, w8vh), (x8h, w8vl), (x8l, w8vh)):
                    for i in range(NP):
                        nc.tensor.matmul(
                            v_ps[:], xa[i][:, :, sc * 128:(sc + 1) * 128],
                            wb[i][:], start=first, stop=False, perf_mode=DR)
                        first = False
                nc.tensor.matmul(v_ps[:], onesbv[:], bv8_t[:],
                                 start=False, stop=True, perf_mode=DR)
                vr = v_ps[:].rearrange("p (h e) -> p h e", h=H_LOC)
                nc.scalar.activation(vhl[sc][:, 0, :, 0:64], vr, Act.Identity)
                nc.vector.scalar_tensor_tensor(
                    vhl[sc][:, 1, :, 0:64], vr, 1.0, vhl[sc][:, 0, :, 0:64],
                    op0=Alu.mult, op1=Alu.subtract)

            # ---- stage D ----
            def emit_scores(h, t):
                s_ps = scps.tile([128, SQ], f32, tag="sc", name=f"sps{h}_{t}")
                hh = h % 2
                for sp in range(NSP):
                    nc.tensor.matmul(
                        s_ps[:, sp * 512:(sp + 1) * 512],
                        kt8[h // 2][32 * hh:32 * hh + 32, :,
                                    t * 128:(t + 1) * 128],
                        qt8[h // 2][32 * hh:32 * hh + 32, :,
                                    sp * 512:(sp + 1) * 512],
                        start=True, stop=True, perf_mode=DR)
                p8t = p8pool.tile([128, SQ], fp8, tag="p8", name=f"p8_{h}_{t}")
                if t in ADD_TCNS:
                    nc.vector.scalar_tensor_tensor(
                        p8t[:].bitcast(u8), s_ps[:], LOG2E, mask_t[t][:],
                        op0=Alu.mult, op1=Alu.add)
                else:
                    e8t = e8pool.tile([128, SQ], fp8, tag="e8",
                                      name=f"e8_{h}_{t}")
                    nc.scalar.activation(e8t[:], s_ps[:], Act.Exp,
                                         scale=0.125, bias=bias5_t[:])
                    eng = nc.vector if (h, t) in DVEMULT else nc.gpsimd
                    eng.tensor_tensor(p8t[:], e8t[:], mask_t[t][:],
                                      op=Alu.mult)
                return p8t

            def emit_av(h, t, p8t, av_t, first):
                for sp in range(NSP):
                    rhs = p8t[:, sp * 512:(sp + 1) * 512].rearrange(
                        "q (two f) -> q two f", two=1).broadcast_to(
                        (128, 2, 512))
                    nc.tensor.matmul(
                        av_t[:, sp * 512:(sp + 1) * 512],
                        vhl[t][:, :, h, :], rhs,
                        start=first, stop=False,
                        perf_mode=DR, skip_group_check=True)

            def emit_delta(h, av_t):
                # runs AFTER all AV chunks: start=True would zero the whole
                # 512-wide psum zero-region, wiping sibling delta blocks.
                for sc in range(8):
                    nc.tensor.matmul(
                        av_t[0:66, sc * 128:(sc + 1) * 128],
                        vhl[sc][:, :, h, :], dd_t[:],
                        start=False, stop=(sc % 4 == 3),
                        perf_mode=DR, skip_group_check=True)

            def emit_epilogue(h, av_t):
                o_t = otpool.tile([65, SQ], f32, tag="ot", name=f"ot{h}")
                nc.scalar.activation(o_t[:], av_t[0:65, :], Act.Identity)
                nc.sync.dma_start(out_d.ap()[h], o_t[:])

            # ---- emission schedule ----
            emit_norms()
            emit_q(0)
            emit_q(1)
            for t in range(6):
                emit_gram(t)
            emit_k(0)
            for sc in range(4):
                emit_v(sc)
            emit_q(2)
            emit_q(3)

            prework = {h: [] for h in range(H_LOC)}
            pre0 = prework[0]
            for t in range(6, NT):
                pre0.append(lambda t=t: emit_gram(t))
                sc = t - 2
                if sc < NT:
                    pre0.append(lambda sc=sc: emit_v(sc))
            pre0.append(lambda: emit_v(14))
            pre0.append(lambda: emit_v(15))
            pre0.append(lambda: emit_k(1))
            prework[1].append(lambda: emit_k(2))
            prework[3].append(lambda: emit_k(3))

            LAG = 6
            for h in range(H_LOC):
                work = prework[h]
                av_t = avps.tile([66, SQ], f32, tag="av", name=f"av{h}")
                pending = []
                for t in range(NT):
                    for _ in range(2):
                        if work:
                            work.pop(0)()
                    p8t = emit_scores(h, t)
                    pending.append((t, p8t))
                    if len(pending) > LAG:
                        tt, pp = pending.pop(0)
                        emit_av(h, tt, pp, av_t, first=(tt == 0))
                while pending:
                    tt, pp = pending.pop(0)
                    emit_av(h, tt, pp, av_t, first=(tt == 0))
                emit_delta(h, av_t)
                emit_epilogue(h, av_t)

    nc.compile()
    return nc


def _get_nc():
    key = (S, D, H_TOT, SIM_THRESH)
    if key not in _CACHE:
        _CACHE[key] = _build()
    return _CACHE[key]


def _to_pairs(a):
    """[D, N] -> [D/256, 128, 2, N] pair layout."""
    d, n = a.shape
    return np.ascontiguousarray(
        a.reshape(d // 256, 2, 128, n).transpose(0, 2, 1, 3))


def make_in_maps(x, Wq, bq, Wk, bk, Wv, bv, h_loc=8, sq=1024, n_cores=N_CORES):
    """Per-core input dicts. Core c: batch, head-group, query-slice; its
    keys are rolled so the query slice comes first. Host work is dtype
    conversion + layout only."""
    import ml_dtypes
    F8 = ml_dtypes.float8_e4m3

    x = np.asarray(x, dtype=np.float32)
    Wq, Wk, Wv = (np.asarray(w, dtype=np.float32) for w in (Wq, Wk, Wv))
    bq, bk, bv = (np.asarray(v_, dtype=np.float32) for v_ in (bq, bk, bv))
    jh = h_loc * DK
    seq, d_model = x.shape[1], x.shape[2]
    n_hg = d_model // jh
    n_qs = seq // sq

    dd = np.zeros((128, 2, 128), np.float32)
    for r in range(128):
        dd[r, 0, r] = DELTA
        dd[r, 1, r] = DELTA
    dd = dd.astype(F8)

    in_maps = []
    for c in range(n_cores):
        b = c // (n_hg * n_qs)
        hg = (c % (n_hg * n_qs)) // n_qs
        qs = c % n_qs
        xb = x[b]
        order = np.concatenate([
            np.arange(qs * sq, (qs + 1) * sq),
            np.delete(np.arange(seq), np.s_[qs * sq:(qs + 1) * sq])])
        xt = np.ascontiguousarray(xb[order].T)          # [D, S]
        xh8 = xt.astype(F8)
        xl8 = (xt - xh8.astype(np.float32)).astype(F8)

        wqt = np.ascontiguousarray(Wq[hg * jh:(hg + 1) * jh].T)
        wkt = np.ascontiguousarray(Wk[hg * jh:(hg + 1) * jh].T)
        wvt = np.ascontiguousarray(Wv[hg * jh:(hg + 1) * jh].T)
        wvh8 = wvt.astype(F8)
        wvl8 = (wvt - wvh8.astype(np.float32)).astype(F8)

        bqs = bq[hg * jh:(hg + 1) * jh]
        bks = bk[hg * jh:(hg + 1) * jh]
        bvs = bv[hg * jh:(hg + 1) * jh]
        bvh8 = bvs.astype(F8)
        bvl8 = (bvs - bvh8.astype(np.float32)).astype(F8)

        in_maps.append({
            "x8h": _to_pairs(xh8),
            "x8l": _to_pairs(xl8),
            "w8q": _to_pairs(wqt.astype(F8)),
            "w8k": _to_pairs(wkt.astype(F8)),
            "w8vh": _to_pairs(wvh8),
            "w8vl": _to_pairs(wvl8),
            "bq128": np.ascontiguousarray(bqs.reshape(4, 128).T),
            "bk128": np.ascontiguousarray(bks.reshape(4, 128).T),
            "bv8": np.ascontiguousarray(
                np.stack([bvh8, bvl8], axis=0)[None]),
            "dd": dd,
        })
    return in_maps


def assemble(results, h_tot=H_TOT, seq=S, h_loc=8, sq=1024, n_cores=N_CORES):
    n_hg = h_tot // h_loc
    n_qs = seq // sq
    n_b = n_cores // (n_hg * n_qs)
    out = np.empty((n_b, h_tot, seq, DK), np.float32)
    for c in range(n_cores):
        b = c // (n_hg * n_qs)
        hg = (c % (n_hg * n_qs)) // n_qs
        qs = c % n_qs
        r = results[c]["out"]                       # [8, 65, SQ]
        att = r[:, 0:64, :] / r[:, 64:65, :]        # host normalize
        out[b, hg * h_loc:(hg + 1) * h_loc, qs * sq:(qs + 1) * sq, :] = \
            att.transpose(0, 2, 1)
    return out


def kernel(x, Wq, bq, Wk, bk, Wv, bv, _trace=False):
    from concourse.bass_utils import run_bass_kernel_spmd
    nc = _get_nc()
    in_maps = make_in_maps(x, Wq, bq, Wk, bk, Wv, bv)
    res = run_bass_kernel_spmd(nc, in_maps, core_ids=list(range(N_CORES)),
                               trace=_trace)
    out = assemble(res.results)
    if _trace:
        return out, res
    return out
